# revision 1
# baseline (speedup 1.0000x reference)
"""Dense 3-layer GAT on 8 TRN2 NeuronCores.

Sharding: each core owns 512 query nodes (rows of the attention score
matrix). Per layer, each core computes h = x @ W and f = x @ (W @ a) for
its own nodes, AllGathers h (bf16) and f (f32) across the 8 cores, then
computes its 512-query slab of masked-softmax attention and the attended
output.

Everything on device is kept transposed (features on partitions, nodes
on the free dim) so layer outputs feed the next layer's matmuls with no
transposes. The h/attention path runs in bf16 (TensorE streams bf16 at
2x the fp32 rate and DVE hits its packed modes); the attention-logit
path (f = x @ (W@a), leaky-relu bias, exp input) stays fp32 since the
softmax is sensitive to absolute errors in the logits.

kernel(**inputs) takes the full unsharded inputs and returns the full
[4096, 256] output.
"""

from contextlib import ExitStack

import numpy as np
import ml_dtypes

import concourse.mybir as mybir
import concourse.tile as tile
from concourse import bacc
from concourse.bass_utils import run_bass_kernel_spmd
from concourse.masks import make_identity

P = 128
N_NODES = 4096
S = 512                    # nodes per core
NB = N_NODES // P          # 32 global key blocks
H = 4
LAYERS = [(512, 512), (2048, 512), (2048, 64)]
F32 = mybir.dt.float32
BF16 = mybir.dt.bfloat16
AF = mybir.ActivationFunctionType
ALU = mybir.AluOpType

_CACHE = {}


def _build():
    nc = bacc.Bacc("TRN2", target_bir_lowering=False, debug=False, num_devices=8)

    xT0_d = nc.dram_tensor("xT0", [512, S], F32, kind="ExternalInput")
    adjT_d = nc.dram_tensor("adjT", [N_NODES, S], BF16, kind="ExternalInput")
    W_d = []
    WA_d = []
    for li, (fin, fout) in enumerate(LAYERS):
        W_d.append(nc.dram_tensor(f"W{li}", [H, fin, fout], BF16, kind="ExternalInput"))
        WA_d.append(nc.dram_tensor(f"WA{li}", [fin, 2 * H], F32, kind="ExternalInput"))
    outT_d = nc.dram_tensor("outT", [H * 64, S], F32, kind="ExternalOutput")

    with tile.TileContext(nc) as tc:
        with ExitStack() as ctx:
            constp = ctx.enter_context(tc.tile_pool(name="const", bufs=1))
            adjp = ctx.enter_context(tc.tile_pool(name="adjp", bufs=1))
            xtp = ctx.enter_context(tc.tile_pool(name="xt", bufs=20))
            xbp = ctx.enter_context(tc.tile_pool(name="xb", bufs=21))
            wtp = ctx.enter_context(tc.tile_pool(name="wt", bufs=17))
            wap = ctx.enter_context(tc.tile_pool(name="wap", bufs=17))
            hfp = ctx.enter_context(tc.tile_pool(name="hfp", bufs=3))
            hgp = ctx.enter_context(tc.tile_pool(name="hgp", bufs=4))
            scp = ctx.enter_context(tc.tile_pool(name="scp", bufs=2))
            fbp = ctx.enter_context(tc.tile_pool(name="fbp", bufs=1))
            fdp = ctx.enter_context(tc.tile_pool(name="fdp", bufs=2))
            frp = ctx.enter_context(tc.tile_pool(name="frp", bufs=3))
            evp = ctx.enter_context(tc.tile_pool(name="evp", bufs=2))
            rcp = ctx.enter_context(tc.tile_pool(name="rcp", bufs=2))
            prp = ctx.enter_context(tc.tile_pool(name="prp", bufs=6))
            psA = ctx.enter_context(tc.tile_pool(name="psA", bufs=2, space="PSUM"))
            psO = ctx.enter_context(tc.tile_pool(name="psO", bufs=4, space="PSUM"))
            psR = ctx.enter_context(tc.tile_pool(name="psR", bufs=1, space="PSUM"))
            dr = ctx.enter_context(tc.tile_pool(name="dram", bufs=1, space="DRAM"))

            ident = constp.tile([P, P], F32, tag="ident")
            make_identity(nc, ident[:])
            # dummy collective to absorb ncfw first-call staging cost while
            # the layer-0 matmuls run
            warm_in = dr.tile([P, 4], F32, tag="warm_in")
            warm_out = dr.tile([8, P, 4], F32, tag="warm_out", addr_space="Shared")
            warm_sb = constp.tile([P, 4], F32, tag="warm_sb", name="warm_sb")
            nc.any.memset(warm_sb[:], 0.0)
            nc.sync.dma_start(warm_in[:], warm_sb[:])
            nc.gpsimd.collective_compute(
                "AllGather", ALU.bypass,
                replica_groups=[list(range(8))],
                ins=[warm_in[:].opt()], outs=[warm_out[:].opt()],
            )
            ones_r = constp.tile([1, P], F32, tag="ones_r")
            nc.any.memset(ones_r[:], 1.0)
            ones_c = constp.tile([P, 1], BF16, tag="ones_c")
            nc.any.memset(ones_c[:], 1.0)

            # resident adjacency (transposed slab), bf16, [key m, own query n]
            adjT_res = adjp.tile([P, NB, S], BF16, tag="adjT")
            nc.sync.dma_start(
                adjT_res[:], adjT_d[:].rearrange("(nb p) n -> p nb n", p=P)
            )

            # layer-0 x^T (own nodes): f32 for the f matmuls + bf16 for h
            xt_cur = []
            xb_cur = []
            for kb in range(4):
                t = xtp.tile([P, S], F32, tag="xt")
                nc.sync.dma_start(t[:], xT0_d[kb * P:(kb + 1) * P, :])
                xt_cur.append(t)
                tb = xbp.tile([P, S], BF16, tag="xb")
                nc.vector.tensor_copy(tb[:], t[:])
                xb_cur.append(tb)

            for li, (fin, fout) in enumerate(LAYERS):
                KB = fin // P
                agh_in = dr.tile([H, S, fout], BF16, tag=f"aghi{li}")
                agh_out = [
                    dr.tile([8, 2, S, fout], BF16, tag=f"agho{li}_{pp}",
                            name=f"agho{li}_{pp}", addr_space="Shared")
                    for pp in range(2)
                ]
                agf_in = dr.tile([2 * H, S], F32, tag=f"agfi{li}")
                agf_out = dr.tile([8, 2 * H, S], F32, tag=f"agfo{li}",
                                  addr_space="Shared")

                # ---- phase A1: f = x @ WA (fp32) ----
                wa_tiles = []
                for kb in range(KB):
                    t = wap.tile([P, 2 * H], F32, tag="wa")
                    nc.sync.dma_start(t[:], WA_d[li][kb * P:(kb + 1) * P, :])
                    wa_tiles.append(t)
                ptf = psA.tile([P, S], F32, tag="ph", name=f"ptf{li}")
                for b in range(4):
                    pf = psA.tile([P, S], F32, tag="ph", name=f"pf{li}_{b}")
                    for kb in range(KB):
                        nc.tensor.matmul(
                            pf[:, 0:2 * H],
                            xt_cur[kb][:, b * P:(b + 1) * P],
                            wa_tiles[kb][:],
                            start=(kb == 0), stop=(kb == KB - 1),
                        )
                    f_sb = hfp.tile([P, 2 * H], F32, tag="fsb")
                    nc.vector.tensor_copy(f_sb[:], pf[:, 0:2 * H])
                    nc.tensor.transpose(
                        ptf[0:2 * H, b * P:(b + 1) * P], f_sb[:], ident[:]
                    )
                fT_loc = hfp.tile([2 * H, S], F32, tag="ftl")
                nc.scalar.copy(fT_loc[:], ptf[0:2 * H, :])
                nc.sync.dma_start(agf_in[:], fT_loc[:])

                # f gather is tiny; issue it before the h matmuls so it hides
                nc.gpsimd.collective_compute(
                    "AllGather", ALU.bypass,
                    replica_groups=[list(range(8))],
                    ins=[agf_in[:].opt()], outs=[agf_out[:].opt()],
                )

                # ---- phase A2: h = x @ W (bf16), shared-weight loop ----
                w_tiles = {}
                for h in range(H):
                    for kb in range(KB):
                        t = wtp.tile([P, fout], BF16, tag="wt", name=f"w{li}_{h}_{kb}")
                        nc.sync.dma_start(t[:], W_d[li][h, kb * P:(kb + 1) * P, :])
                        w_tiles[(h, kb)] = t
                for h in range(H):
                    for b in range(4):
                        ph = psA.tile([P, S], F32, tag="ph", name=f"ph{li}_{h}_{b}")
                        for kb in range(KB):
                            nc.tensor.matmul(
                                ph[:, 0:fout],
                                xb_cur[kb][:, b * P:(b + 1) * P],
                                w_tiles[(h, kb)][:],
                                start=(kb == 0), stop=(kb == KB - 1),
                            )
                        h_sb = hfp.tile([P, fout], BF16, tag="hsb")
                        nc.vector.tensor_copy(h_sb[:], ph[:, 0:fout])
                        nc.gpsimd.dma_start(agh_in[h, b * P:(b + 1) * P, :], h_sb[:])
                    # per-head-pair gather overlaps the next heads' matmuls
                    if h % 2 == 1:
                        nc.gpsimd.collective_compute(
                            "AllGather", ALU.bypass,
                            replica_groups=[list(range(8))],
                            ins=[agh_in[h - 1:h + 1].opt()],
                            outs=[agh_out[h // 2][:].opt()],
                        )

                # ---- phase C: f_src broadcast + f_dst layout ----
                fsb_bcast = fbp.tile([P, H, S], F32, tag="fsb_b")
                for h in range(H):
                    fr = frp.tile([1, S], F32, tag="fr")
                    nc.sync.dma_start(fr[:], agf_in[2 * h:2 * h + 1, :])
                    pb = psA.tile([P, S], F32, tag="ph", name=f"pb{li}_{h}")
                    nc.tensor.matmul(pb[:], ones_r[:], fr[:], start=True, stop=True)
                    nc.scalar.copy(fsb_bcast[:, h, :], pb[:])
                fgat = fdp.tile([64, S], F32, tag="fgat")
                nc.sync.dma_start(fgat[:], agf_out[:].rearrange("r j m -> (r j) m"))
                ptd = psA.tile([P, S], F32, tag="ph", name=f"ptd{li}")
                for c in range(4):
                    nc.tensor.transpose(
                        ptd[:, c * 64:(c + 1) * 64],
                        fgat[:, c * P:(c + 1) * P],
                        ident[0:64, 0:64],
                    )
                # fT_sb[ml, mh, r, j] = f[j][r*512 + mh*128 + ml]
                fT_sb = fdp.tile([P, 4, 8, 2 * H], F32, tag="fdst")
                nc.scalar.copy(
                    fT_sb[:], ptd[:, 0:256].rearrange("p (mh rj) -> p mh rj", mh=4)
                    .rearrange("p mh (r j) -> p mh r j", r=8)
                )

                # ---- phase D: attention ----
                xt_next = []
                xb_next = []
                nob = 1 if fout == 64 else 4
                pending_evict = None
                for h in range(H):
                    po = [
                        psO.tile([P, S], F32, tag="po", name=f"po{li}_{h}_{ob}")
                        for ob in range(nob)
                    ]
                    prs = psR.tile([1, S], F32, tag="prs", name=f"prs{li}_{h}")
                    for mbg in range(NB // 2):
                        lr4 = scp.tile([P, 2, S], F32, tag="lr")
                        for i in range(2):
                            nc.scalar.activation(
                                lr4[:, i, :], fsb_bcast[:, h, :], AF.Prelu,
                                bias=fT_sb[:, (2 * mbg + i) % 4, (2 * mbg + i) // 4, 2 * h + 1:2 * h + 2],
                                scale=1.0, alpha=0.2,
                            )
                        ex4 = scp.tile([P, 2, S], BF16, tag="ex")
                        nc.scalar.activation(ex4[:], lr4[:], AF.Exp, bias=0.0, scale=1.0)
                        st4 = scp.tile([P, 2, S], BF16, tag="st")
                        nc.vector.tensor_tensor(
                            st4[:], ex4[:], adjT_res[:, 2 * mbg:2 * mbg + 2, :], ALU.mult
                        )
                        for i in range(2):
                            mb = 2 * mbg + i
                            r, bsub = mb // 4, mb % 4
                            s_t = st4[:, i, :]
                            hg = hgp.tile([P, fout], BF16, tag="hg")
                            nc.gpsimd.dma_start(
                                hg[:], agh_out[h // 2][r, h % 2, bsub * P:(bsub + 1) * P, :]
                            )
                            for ob in range(nob):
                                nc.tensor.matmul(
                                    po[ob][:, :] if fout != 64 else po[ob][0:64, :],
                                    hg[:, ob * P:(ob + 1) * P] if fout != 64 else hg[:],
                                    s_t,
                                    start=(mb == 0), stop=(mb == NB - 1),
                                )
                            nc.tensor.matmul(
                                prs[:], ones_c[:], s_t,
                                start=(mb == 0), stop=(mb == NB - 1),
                            )

                    # free the PSUM banks promptly (ACT copies), defer the
                    # DVE normalize+elu so the next head's score TTs are not
                    # queued behind a premature PE wait on the vector engine
                    if pending_evict is not None:
                        pending_evict()
                        pending_evict = None
                    rows = 64 if fout == 64 else P
                    praw = []
                    for ob in range(nob):
                        src = po[ob][0:64, :] if fout == 64 else po[ob][:]
                        pr_sb = prp.tile([rows, S], F32, tag="praw", name=f"praw{li}_{h}_{ob}")
                        nc.scalar.copy(pr_sb[:], src)
                        praw.append(pr_sb)
                    rsum = rcp.tile([1, S], F32, tag="rsum")
                    nc.scalar.copy(rsum[:], prs[:])
                    pb2 = psA.tile([P, S], F32, tag="ph", name=f"pb2{li}_{h}")
                    nc.tensor.matmul(pb2[:], ones_r[:], rsum[:], start=True, stop=True)
                    rb0 = rcp.tile([P, S], F32, tag="rb0")
                    nc.scalar.copy(rb0[:], pb2[:])

                    def _evict(praw=praw, rb0=rb0, li=li, rows=rows):
                        rb = rcp.tile([P, S], F32, tag="rb", name=f"rb{li}")
                        nc.vector.reciprocal_approx_fast(rb[:], rb0[:])
                        for pr_sb in praw:
                            t0 = evp.tile([rows, S], F32, tag="t0", name=f"t0{li}")
                            nc.vector.tensor_tensor(t0[:], pr_sb[:], rb[0:rows, :], ALU.mult)
                            # elu(x) = min(exp(x) - 1, relu(x))
                            em = evp.tile([rows, S], F32, tag="em", name=f"em{li}")
                            nc.scalar.activation(em[:], t0[:], AF.Exp, bias=0.0, scale=1.0)
                            rl = evp.tile([rows, S], F32, tag="rl", name=f"rl{li}")
                            nc.vector.tensor_scalar_max(rl[:], t0[:], 0.0)
                            xnt = xtp.tile([rows, S], F32, tag="xt", name=f"xt{li}")
                            nc.vector.scalar_tensor_tensor(
                                xnt[:], em[:], -1.0, rl[:], ALU.add, ALU.min
                            )
                            if li == 2:
                                em2 = evp.tile([rows, S], F32, tag="em", name=f"em2{li}")
                                nc.scalar.activation(em2[:], xnt[:], AF.Exp, bias=0.0, scale=1.0)
                                rl2 = evp.tile([rows, S], F32, tag="rl", name=f"rl2{li}")
                                nc.vector.tensor_scalar_max(rl2[:], xnt[:], 0.0)
                                x2 = xtp.tile([rows, S], F32, tag="xt", name=f"x2{li}")
                                nc.vector.scalar_tensor_tensor(
                                    x2[:], em2[:], -1.0, rl2[:], ALU.add, ALU.min
                                )
                                xnt = x2
                            xt_next.append(xnt)
                            if li < 2:
                                xbn = xbp.tile([rows, S], BF16, tag="xb", name=f"xb{li}")
                                nc.vector.tensor_copy(xbn[:], xnt[:])
                                xb_next.append(xbn)

                    pending_evict = _evict

                if pending_evict is not None:
                    pending_evict()
                    pending_evict = None

                xt_cur = xt_next
                xb_cur = xb_next

            # final output: xt_cur is 4 tiles of [64, 512] (head-major)
            for h in range(H):
                nc.sync.dma_start(outT_d[h * 64:(h + 1) * 64, :], xt_cur[h][:])

    nc.compile()
    return nc


def build_in_maps(inputs):
    node_feats = np.ascontiguousarray(inputs["node_feats"], dtype=np.float32)
    adj = np.asarray(inputs["adj"], dtype=np.float32)
    Ws = [np.asarray(inputs[f"W{i}"], dtype=np.float32) for i in range(3)]
    As = [np.asarray(inputs[f"a{i}"], dtype=np.float32) for i in range(3)]

    WAs = []
    for W, a in zip(Ws, As):
        wa = np.einsum(
            "hfo,hjo->fhj", W.astype(np.float64), a.astype(np.float64)
        ).reshape(W.shape[1], 2 * H).astype(np.float32)
        WAs.append(np.ascontiguousarray(wa))
    Wbf = [W.astype(ml_dtypes.bfloat16) for W in Ws]

    in_maps = []
    for c in range(8):
        rows = slice(c * S, (c + 1) * S)
        m = {
            "xT0": np.ascontiguousarray(node_feats[rows].T),
            "adjT": np.ascontiguousarray(adj[rows].T).astype(ml_dtypes.bfloat16),
        }
        for i in range(3):
            m[f"W{i}"] = Wbf[i]
            m[f"WA{i}"] = WAs[i]
        in_maps.append(m)
    return in_maps


def kernel(**inputs):
    if "nc" not in _CACHE:
        _CACHE["nc"] = _build()
    nc = _CACHE["nc"]
    in_maps = build_in_maps(inputs)
    res = run_bass_kernel_spmd(nc, in_maps, core_ids=list(range(8)))
    out = np.concatenate([r["outT"].T for r in res.results], axis=0)
    return np.ascontiguousarray(out, dtype=np.float32)


if __name__ == "__main__":
    rng = np.random.default_rng(0)
    fake = {
        "node_feats": rng.standard_normal((N_NODES, 512), dtype=np.float32),
        "edge_feats": rng.standard_normal((131072, 16), dtype=np.float32),
        "edge_indices": rng.integers(0, N_NODES, (2, 131072)).astype(np.int32),
        "adj": np.maximum(
            (rng.random((N_NODES, N_NODES)) < 0.01).astype(np.float32),
            np.eye(N_NODES, dtype=np.float32),
        ),
    }
    for i, (fin, fout) in enumerate(LAYERS):
        fake[f"W{i}"] = (rng.standard_normal((H, fin, fout)) * 0.05).astype(np.float32)
        fake[f"a{i}"] = (rng.standard_normal((H, 2, fout)) * 0.05).astype(np.float32)
    o = kernel(**fake)
    print("kernel output", o.shape, o.dtype, np.abs(o).mean())



# revision 23
# speedup vs baseline: 1.3304x; 1.3304x over previous
"""Dense 3-layer GAT on 8 TRN2 NeuronCores (v2).

Sharding: each core owns 512 query nodes (rows of the attention score
matrix). Per layer, each core computes h = x @ W for its own nodes,
AllGathers h (bf16), then computes its 512-query slab of masked-softmax
attention and the attended output.

v2 structure (vs v1):
- adjacency mask folded into the logits as an additive -30000 logmask,
  applied during the score computation on DVE (2 DVE ops + 1 ACT exp
  per 128x512 block); no separate mask multiply.
- softmax denominators via an all-ones [128,128] stationary matmul that
  accumulates broadcast row-sums in PSUM (L0/L1); for L2 a ones column
  is appended to the gathered h so the row-sum falls out of the same
  matmul that computes the attended output.
- f = x @ (W@a) computed with WA as the stationary operand (out [8,S]
  directly in f^T layout, bf16); layer 0 computes f for ALL nodes
  locally from a replicated full x (no layer-0 f AllGather).
- layer boundaries software-pipelined: next layer's f/h matmuls are
  emitted right after the last head's eviction with the AllGathers
  interleaved, so gather latency hides under matmul streams.
- hg tiles are prefetched on the sync DMA queue several key-blocks
  ahead; normalization reads attention PSUM directly.
"""

from contextlib import ExitStack

import numpy as np
import ml_dtypes

import concourse.mybir as mybir
import concourse.tile as tile
from concourse import bacc
from concourse.bass_utils import run_bass_kernel_spmd
from concourse.masks import make_identity

P = 128
N_NODES = 4096
S = 512                    # nodes per core
NB = N_NODES // P          # 32 global key blocks
H = 4
J = 2 * H                  # f rows (src/dst per head)
LAYERS = [(512, 512), (2048, 512), (2048, 64)]
F32 = mybir.dt.float32
BF16 = mybir.dt.bfloat16
AF = mybir.ActivationFunctionType
ALU = mybir.AluOpType
NEG = -30000.0
HG_AHEAD = 4               # hg prefetch depth (key blocks)

_CACHE = {}


def _build():
    nc = bacc.Bacc("TRN2", target_bir_lowering=False, debug=False, num_devices=8)

    x0own_d = nc.dram_tensor("x0own", [512, S], BF16, kind="ExternalInput")
    f0T_d = nc.dram_tensor("f0T", [64, S], F32, kind="ExternalInput")
    f0own_d = nc.dram_tensor("f0own", [J, S], BF16, kind="ExternalInput")
    lmask_d = nc.dram_tensor("lmaskT", [N_NODES, S], BF16, kind="ExternalInput")
    W_d = []
    WA_d = []
    for li, (fin, fout) in enumerate(LAYERS):
        wshape = [H, fin, fout] if li < 2 else [fin, H * 64]
        W_d.append(nc.dram_tensor(f"W{li}", wshape, BF16, kind="ExternalInput"))
        WA_d.append(nc.dram_tensor(f"WA{li}", [fin, J], BF16, kind="ExternalInput"))
    outT_d = nc.dram_tensor("outT", [H * 64, S], F32, kind="ExternalOutput")

    with tile.TileContext(nc) as tc:
        with ExitStack() as ctx:
            constp = ctx.enter_context(tc.tile_pool(name="const", bufs=1))
            lmp = ctx.enter_context(tc.tile_pool(name="lmp", bufs=1))
            x0p = ctx.enter_context(tc.tile_pool(name="x0p", bufs=8))
            xop = ctx.enter_context(tc.tile_pool(name="xop", bufs=4))
            xbp = ctx.enter_context(tc.tile_pool(name="xbp", bufs=32))
            wtp = ctx.enter_context(tc.tile_pool(name="wt", bufs=17))
            wap = ctx.enter_context(tc.tile_pool(name="wap", bufs=17))
            hsp = ctx.enter_context(tc.tile_pool(name="hsp", bufs=4))
            hgp = ctx.enter_context(tc.tile_pool(name="hgp", bufs=8))
            fp = ctx.enter_context(tc.tile_pool(name="fp", bufs=2))
            fsrcp = ctx.enter_context(tc.tile_pool(name="fsrcp", bufs=5))
            zp = ctx.enter_context(tc.tile_pool(name="zp", bufs=4))
            stp = ctx.enter_context(tc.tile_pool(name="stp", bufs=6))
            rcp = ctx.enter_context(tc.tile_pool(name="rcp", bufs=2))
            evp = ctx.enter_context(tc.tile_pool(name="evp", bufs=2))
            psO = ctx.enter_context(tc.tile_pool(name="psO", bufs=4, space="PSUM"))
            psR = ctx.enter_context(tc.tile_pool(name="psR", bufs=2, space="PSUM"))
            psA = ctx.enter_context(tc.tile_pool(name="psA", bufs=2, space="PSUM"))
            dr = ctx.enter_context(tc.tile_pool(name="dram", bufs=1, space="DRAM"))

            identb = constp.tile([64, 64], F32, tag="identb")
            make_identity(nc, identb[:])
            ones_r = constp.tile([1, P], BF16, tag="ones_r")
            nc.any.memset(ones_r[:], 1.0)
            ones_rf = constp.tile([1, P], F32, tag="ones_rf")
            nc.any.memset(ones_rf[:], 1.0)
            ones128 = constp.tile([P, P], BF16, tag="ones128")
            nc.any.memset(ones128[:], 1.0)

            # warmup collective to absorb ncfw first-call staging; emitted
            # first so it completes before the first real gather is needed
            warm_in = dr.tile([P, 4], F32, tag="warm_in")
            warm_out = dr.tile([8, P, 4], F32, tag="warm_out", addr_space="Shared")
            warm_sb = constp.tile([P, 4], F32, tag="warm_sb")
            nc.any.memset(warm_sb[:], 0.0)
            nc.sync.dma_start(warm_in[:], warm_sb[:])
            nc.gpsimd.collective_compute(
                "AllGather", ALU.bypass,
                replica_groups=[list(range(8))],
                ins=[warm_in[:].opt()], outs=[warm_out[:].opt()],
            )

            # resident logmask (transposed slab), bf16, [key m, own query n]
            lmask_res = lmp.tile([P, NB, S], BF16, tag="lmask")
            nc.sync.dma_start(
                lmask_res[:], lmask_d[:].rearrange("(nb p) n -> p nb n", p=P)
            )

            # layer-0 own x (bf16), 4 fin-chunks [128, 512]
            x0own = []
            for kb in range(4):
                t = xop.tile([P, S], BF16, tag="x0own")
                nc.sync.dma_start(t[:], x0own_d[kb * P:(kb + 1) * P, :])
                x0own.append(t)

            # shared DRAM buffers for the gathers
            # L0: per-head single gathers; L1/L2: per-pair gathers
            agh0_in = [
                dr.tile([S, 512], BF16, tag=f"ag0i{h}", name=f"ag0i{h}")
                for h in range(H)
            ]
            agh0_out = [
                dr.tile([8, S, 512], BF16, tag=f"ag0o{h}", name=f"ag0o{h}",
                        addr_space="Shared")
                for h in range(H)
            ]
            agh_in = {}
            agh_out = {}
            agf_in = {}
            agf_out = {}
            for li in (1, 2):
                fout = LAYERS[li][1]
                for pp in range(2):
                    agh_in[(li, pp)] = dr.tile(
                        [2, S, fout], BF16, tag=f"aghi{li}_{pp}",
                        name=f"aghi{li}_{pp}")
                    agh_out[(li, pp)] = dr.tile(
                        [8, 2, S, fout], BF16, tag=f"agho{li}_{pp}",
                        name=f"agho{li}_{pp}", addr_space="Shared")
                agf_in[li] = dr.tile([J, S], F32, tag=f"agfi{li}",
                                     name=f"agfi{li}")
                agf_out[li] = dr.tile([8, J, S], F32, tag=f"agfo{li}",
                                      name=f"agfo{li}", addr_space="Shared")

            # L2 hg tiles with a preset ones column (col 64)
            hg65 = []
            for i in range(6):
                t = hgp.tile([P, 65], BF16, tag=f"hg65_{i}")
                nc.any.memset(t[:, 64:65], 1.0)
                hg65.append(t)

            def load_w_tiles(li):
                fin, fout = LAYERS[li]
                KB = fin // P
                w = {}
                if li < 2:
                    for h in range(H):
                        for kb in range(KB):
                            t = wtp.tile([P, fout], BF16, tag="wt",
                                         name=f"w{li}_{h}_{kb}")
                            nc.sync.dma_start(
                                t[:], W_d[li][h, kb * P:(kb + 1) * P, :])
                            w[(h, kb)] = t
                else:
                    for kb in range(KB):
                        t = wtp.tile([P, H * 64], BF16, tag="wt2",
                                     name=f"w{li}_{kb}")
                        nc.sync.dma_start(t[:], W_d[li][kb * P:(kb + 1) * P, :])
                        w[kb] = t
                return w

            def load_wa_tiles(li):
                fin = LAYERS[li][0]
                KB = fin // P
                tiles = []
                for kb in range(KB):
                    t = wap.tile([P, J], BF16, tag="wa", name=f"wa{li}_{kb}")
                    nc.sync.dma_start(t[:], WA_d[li][kb * P:(kb + 1) * P, :])
                    tiles.append(t)
                return tiles

            # ---------------- layer 0 front end ----------------
            w0 = load_w_tiles(0)

            def l0_head_mm(h):
                for b in range(4):
                    ph = psA.tile([P, S], F32, tag="pa", name=f"ph0_{h}_{b}")
                    for kb in range(4):
                        nc.tensor.matmul(
                            ph[:, 0:512],
                            x0own[kb][:, b * P:(b + 1) * P],
                            w0[(h, kb)][:],
                            start=(kb == 0), stop=(kb == 3),
                        )
                    hsb = hsp.tile([P, 512], BF16, tag="hsb")
                    nc.scalar.copy(hsb[:], ph[:, 0:512])
                    nc.sync.dma_start(agh0_in[h][b * P:(b + 1) * P, :], hsb[:])

            def l0_gather(h):
                nc.gpsimd.collective_compute(
                    "AllGather", ALU.bypass,
                    replica_groups=[list(range(8))],
                    ins=[agh0_in[h][:].opt()], outs=[agh0_out[h][:].opt()],
                )

            # layer-0 f computed host-side; load both layouts
            fsb_own0 = fp.tile([J, S], BF16, tag="fsb8", name="fsb_own0")
            nc.sync.dma_start(fsb_own0[:], f0own_d[:])
            fgat0 = fp.tile([64, S], F32, tag="fgat", name="fgat0")
            nc.sync.dma_start(fgat0[:], f0T_d[:])

            # head h-matmuls, each head's gather starts ASAP
            for h in range(H):
                l0_head_mm(h)
                l0_gather(h)

            def fdst_prep(li, fgat_src):
                """fgat_src: [64, 512] bf16 tile view (r-major, j-minor rows).
                Returns fT_sb [128, 4, 8, J] f32 with
                fT_sb[ml, mh, r, j] = f[j][r*512 + mh*128 + ml]."""
                ptd = psA.tile([P, S], F32, tag="pa", name=f"ptd{li}")
                for mh in range(4):
                    nc.tensor.transpose(
                        ptd[:, mh * 64:(mh + 1) * 64],
                        fgat_src[:, mh * P:(mh + 1) * P],
                        identb[:],
                    )
                fT_sb = fp.tile([P, 4, 8, J], F32, tag="fdst", name=f"fdst{li}")
                nc.scalar.copy(
                    fT_sb[:], ptd[:, 0:256]
                    .rearrange("p (mh rj) -> p mh rj", mh=4)
                    .rearrange("p mh (r j) -> p mh r j", r=8)
                )
                return fT_sb

            def fsrc_bcast(li, fsb8, h):
                fr = fp.tile([1, S], BF16, tag="fr", name=f"fr{li}_{h}")
                nc.sync.dma_start(fr[:], fsb8[2 * h:2 * h + 1, :])
                pb = psA.tile([P, S], F32, tag="pa", name=f"pb{li}_{h}")
                nc.tensor.matmul(pb[:], ones_r[:], fr[:],
                                 start=True, stop=True)
                t = fsrcp.tile([P, S], BF16, tag="fsrcb", name=f"fsrcb{li}_{h}")
                nc.scalar.copy(t[:], pb[:])
                return t

            fT_sb0 = fdst_prep(0, fgat0)
            fsrcb0 = [fsrc_bcast(0, fsb_own0, h) for h in range(H)]

            # ---------------- per-layer attention + next-layer front end ----
            def hg_load(li, h, mb):
                """Issue DMA for gathered h tile of key block mb, head h."""
                r, bsub = mb // 4, mb % 4
                if li == 0:
                    t = hgp.tile([P, 512], BF16, tag="hg", name=f"hg{li}")
                    nc.sync.dma_start(
                        t[:], agh0_out[h][r, bsub * P:(bsub + 1) * P, :])
                elif li == 1:
                    t = hgp.tile([P, 512], BF16, tag="hg", name=f"hg{li}")
                    nc.sync.dma_start(
                        t[:],
                        agh_out[(1, h // 2)][r, h % 2, bsub * P:(bsub + 1) * P, :])
                else:
                    t = hg65[(h * NB + mb) % 6]
                    nc.sync.dma_start(
                        t[:, 0:64],
                        agh_out[(2, h // 2)][r, h % 2, bsub * P:(bsub + 1) * P, :])
                return t

            def scores_and_po(li, h, fsrcb, fT_sb):
                """Emit score pipeline + attention matmuls for one head.
                Returns (po_tiles, psr_or_None)."""
                fout = LAYERS[li][1]
                nob = 4 if fout == 512 else 1
                po = [
                    psO.tile([P, S], F32, tag="po", name=f"po{li}_{h}_{ob}")
                    for ob in range(nob)
                ]
                if li < 2:
                    prs = psR.tile([P, S], F32, tag="prs", name=f"prs{li}_{h}")
                else:
                    prs = None
                hgq = [hg_load(li, h, mb) for mb in range(HG_AHEAD)]
                for mbg in range(NB // 2):
                    lrp = zp.tile([P, 2, S], BF16, tag="lr")
                    for i in range(2):
                        mb = 2 * mbg + i
                        z = zp.tile([P, S], BF16, tag="z")
                        nc.vector.scalar_tensor_tensor(
                            z[:], fsrcb[:],
                            fT_sb[:, mb % 4, mb // 4, 2 * h + 1:2 * h + 2],
                            lmask_res[:, mb, :],
                            ALU.add, ALU.add,
                        )
                        nc.vector.scalar_tensor_tensor(
                            lrp[:, i, :], z[:], 0.2, z[:], ALU.mult, ALU.max,
                        )
                    st = stp.tile([P, 2, S], BF16, tag="st")
                    nc.scalar.activation(st[:], lrp[:], AF.Exp, bias=0.0, scale=1.0)
                    for i in range(2):
                        mb = 2 * mbg + i
                        if mb + HG_AHEAD < NB:
                            hgq.append(hg_load(li, h, mb + HG_AHEAD))
                        hg = hgq[mb]
                        s_t = st[:, i, :]
                        if li < 2:
                            for ob in range(nob):
                                nc.tensor.matmul(
                                    po[ob][:, :],
                                    hg[:, ob * P:(ob + 1) * P],
                                    s_t,
                                    start=(mb == 0), stop=(mb == NB - 1),
                                )
                            nc.tensor.matmul(
                                prs[:], ones128[:], s_t,
                                start=(mb == 0), stop=(mb == NB - 1),
                            )
                        else:
                            nc.tensor.matmul(
                                po[0][0:65, :], hg[:], s_t,
                                start=(mb == 0), stop=(mb == NB - 1),
                            )
                return po, prs

            def evict(li, h, po, prs, xb_next):
                """Normalize + ELU for one head; appends bf16 tiles to
                xb_next (L0/L1) or DMAs the final output (L2)."""
                fout = LAYERS[li][1]
                if li < 2:
                    rb = rcp.tile([P, S], F32, tag="rb", name=f"rb{li}_{h}")
                    nc.vector.reciprocal_approx_fast(rb[:], prs[:])
                    for ob in range(4):
                        t0 = evp.tile([P, S], F32, tag="t0")
                        nc.vector.tensor_tensor(t0[:], po[ob][:], rb[:], ALU.mult)
                        em = evp.tile([P, S], F32, tag="em")
                        nc.scalar.activation(em[:], t0[:], AF.Exp, bias=0.0,
                                             scale=1.0)
                        rl = evp.tile([P, S], F32, tag="rl")
                        nc.scalar.activation(rl[:], t0[:], AF.Relu, bias=0.0,
                                             scale=1.0)
                        xbn = xbp.tile([P, S], BF16, tag="xb", name=f"xb{li}")
                        nc.vector.scalar_tensor_tensor(
                            xbn[:], em[:], -1.0, rl[:], ALU.add, ALU.min)
                        xb_next.append(xbn)
                else:
                    # row 64 of po holds the row-sums
                    rs = rcp.tile([1, S], F32, tag="rs2", name=f"rs2_{h}")
                    nc.scalar.copy(rs[:], po[0][64:65, :])
                    rsr = rcp.tile([1, S], F32, tag="rsr2", name=f"rsr2_{h}")
                    nc.vector.reciprocal_approx_fast(rsr[:], rs[:])
                    pbr = psA.tile([P, S], F32, tag="pa", name=f"pbr2_{h}")
                    nc.tensor.matmul(pbr[0:64, :], ones_rf[:, 0:64], rsr[:],
                                     start=True, stop=True)
                    rbs = rcp.tile([64, S], F32, tag="rbs2", name=f"rbs2_{h}")
                    nc.scalar.copy(rbs[:], pbr[0:64, :])
                    t0 = evp.tile([64, S], F32, tag="t02", name="t02")
                    nc.vector.tensor_tensor(t0[:], po[0][0:64, :], rbs[:],
                                            ALU.mult)
                    em = evp.tile([64, S], F32, tag="em2", name="em2")
                    nc.scalar.activation(em[:], t0[:], AF.Exp, bias=0.0, scale=1.0)
                    rl = evp.tile([64, S], F32, tag="rl2", name="rl2")
                    nc.scalar.activation(rl[:], t0[:], AF.Relu, bias=0.0, scale=1.0)
                    x1 = evp.tile([64, S], F32, tag="x12", name="x12")
                    nc.vector.scalar_tensor_tensor(
                        x1[:], em[:], -1.0, rl[:], ALU.add, ALU.min)
                    em2 = evp.tile([64, S], F32, tag="em2", name="em2b")
                    nc.scalar.activation(em2[:], x1[:], AF.Exp, bias=0.0, scale=1.0)
                    rl2 = evp.tile([64, S], F32, tag="rl2", name="rl2b")
                    nc.scalar.activation(rl2[:], x1[:], AF.Relu, bias=0.0,
                                         scale=1.0)
                    x2 = evp.tile([64, S], F32, tag="x12", name="x2b")
                    nc.vector.scalar_tensor_tensor(
                        x2[:], em2[:], -1.0, rl2[:], ALU.add, ALU.min)
                    nc.sync.dma_start(outT_d[h * 64:(h + 1) * 64, :], x2[:])

            def next_front_end(li, xb_cur):
                """f + h matmuls and gathers for layer li (1 or 2), reading
                xb_cur (16 bf16 [128,512] fin-chunk tiles). Returns
                (fsb8, w tiles) for later use."""
                fin, fout = LAYERS[li]
                KB = fin // P
                wa = load_wa_tiles(li)
                w = load_w_tiles(li)
                # f (stationary = WA chunks, stream x) -> psum [8, S]
                psf = psA.tile([P, S], F32, tag="pa", name=f"psf{li}")
                for kb in range(KB):
                    nc.tensor.matmul(
                        psf[0:J, :], wa[kb][:], xb_cur[kb][:],
                        start=(kb == 0), stop=(kb == KB - 1),
                    )
                fsb8 = fp.tile([J, S], F32, tag="fsb8f", name=f"fsb8_{li}")
                nc.scalar.copy(fsb8[:], psf[0:J, :])
                nc.sync.dma_start(agf_in[li][:], fsb8[:])
                fsb8b = fp.tile([J, S], BF16, tag="fsb8", name=f"fsb8b_{li}")
                nc.vector.tensor_copy(fsb8b[:], fsb8[:])
                nc.gpsimd.collective_compute(
                    "AllGather", ALU.bypass,
                    replica_groups=[list(range(8))],
                    ins=[agf_in[li][:].opt()], outs=[agf_out[li][:].opt()],
                )
                # h matmuls, gathers per head pair
                if li == 1:
                    for h in range(H):
                        for b in range(4):
                            ph = psA.tile([P, S], F32, tag="pa",
                                          name=f"ph{li}_{h}_{b}")
                            for kb in range(KB):
                                nc.tensor.matmul(
                                    ph[:, 0:fout],
                                    xb_cur[kb][:, b * P:(b + 1) * P],
                                    w[(h, kb)][:],
                                    start=(kb == 0), stop=(kb == KB - 1),
                                )
                            hsb = hsp.tile([P, fout], BF16, tag="hsb")
                            nc.scalar.copy(hsb[:], ph[:, 0:fout])
                            nc.sync.dma_start(
                                agh_in[(li, h // 2)][h % 2,
                                                     b * P:(b + 1) * P, :],
                                hsb[:])
                        if h % 2 == 1:
                            nc.gpsimd.collective_compute(
                                "AllGather", ALU.bypass,
                                replica_groups=[list(range(8))],
                                ins=[agh_in[(li, h // 2)][:].opt()],
                                outs=[agh_out[(li, h // 2)][:].opt()],
                            )
                else:
                    # L2: all 4 heads in one 256-wide stream per (b, kb)
                    for b in range(4):
                        ph = psA.tile([P, S], F32, tag="pa", name=f"ph2_{b}")
                        for kb in range(KB):
                            nc.tensor.matmul(
                                ph[:, 0:256],
                                xb_cur[kb][:, b * P:(b + 1) * P],
                                w[kb][:],
                                start=(kb == 0), stop=(kb == KB - 1),
                            )
                        hsb = hsp.tile([P, 256], BF16, tag="hsb2")
                        nc.scalar.copy(hsb[:], ph[:, 0:256])
                        for h in range(H):
                            nc.sync.dma_start(
                                agh_in[(2, h // 2)][h % 2,
                                                    b * P:(b + 1) * P, :],
                                hsb[:, h * 64:(h + 1) * 64])
                    for pp in range(2):
                        nc.gpsimd.collective_compute(
                            "AllGather", ALU.bypass,
                            replica_groups=[list(range(8))],
                            ins=[agh_in[(2, pp)][:].opt()],
                            outs=[agh_out[(2, pp)][:].opt()],
                        )
                return fsb8b

            def layer_attention(li, fsrcb, fT_sb, xb_cur):
                """Software-pipelined heads: S(0) S(1) E(0) S(2) E(1) S(3)
                E(2) E(3); returns xb_next. After E(3) the caller emits the
                next layer's front end."""
                xb_next = []
                pend = []
                for h in range(H):
                    po, prs = scores_and_po(li, h, fsrcb[h], fT_sb)
                    pend.append((h, po, prs))
                    if len(pend) >= 2:
                        hh, ppo, pprs = pend.pop(0)
                        evict(li, hh, ppo, pprs, xb_next)
                while pend:
                    hh, ppo, pprs = pend.pop(0)
                    evict(li, hh, ppo, pprs, xb_next)
                return xb_next

            # ---- run the three layers ----
            xb1 = layer_attention(0, fsrcb0, fT_sb0, None)

            fsb8_1 = next_front_end(1, xb1)
            fgat1 = fp.tile([64, S], F32, tag="fgat", name="fgat1")
            nc.sync.dma_start(
                fgat1[:], agf_out[1][:].rearrange("r j m -> (r j) m"))
            fT_sb1 = fdst_prep(1, fgat1)
            fsrcb1 = [fsrc_bcast(1, fsb8_1, h) for h in range(H)]
            xb2 = layer_attention(1, fsrcb1, fT_sb1, xb1)

            fsb8_2 = next_front_end(2, xb2)
            fgat2 = fp.tile([64, S], F32, tag="fgat", name="fgat2")
            nc.sync.dma_start(
                fgat2[:], agf_out[2][:].rearrange("r j m -> (r j) m"))
            fT_sb2 = fdst_prep(2, fgat2)
            fsrcb2 = [fsrc_bcast(2, fsb8_2, h) for h in range(H)]
            layer_attention(2, fsrcb2, fT_sb2, xb2)

    nc.compile()
    return nc


def build_in_maps(inputs):
    node_feats = np.asarray(inputs["node_feats"], dtype=np.float32)
    adj = np.asarray(inputs["adj"], dtype=np.float32)
    Ws = [np.asarray(inputs[f"W{i}"], dtype=np.float32) for i in range(3)]
    As = [np.asarray(inputs[f"a{i}"], dtype=np.float32) for i in range(3)]

    WAs = []
    WAs64 = []
    for W, a in zip(Ws, As):
        wa64 = np.einsum(
            "hfo,hjo->fhj", W.astype(np.float64), a.astype(np.float64)
        ).reshape(W.shape[1], J)
        WAs64.append(wa64)
        WAs.append(np.ascontiguousarray(wa64.astype(ml_dtypes.bfloat16)))
    Wbf = [Ws[0].astype(ml_dtypes.bfloat16), Ws[1].astype(ml_dtypes.bfloat16)]
    # L2 weights merged across heads: [fin, H*64]
    W2m = np.ascontiguousarray(
        np.transpose(Ws[2], (1, 0, 2)).reshape(Ws[2].shape[1], H * 64)
    ).astype(ml_dtypes.bfloat16)

    x0T = np.ascontiguousarray(node_feats.T).astype(ml_dtypes.bfloat16)
    # layer-0 f = x @ (W0@a0) for all nodes, in the gather layout
    # f0T[(r*8+j), c] = f0[r*512+c, j]
    f0 = (node_feats.astype(np.float64) @ WAs64[0]).astype(np.float32)  # [N, J]
    f0T = np.ascontiguousarray(
        f0.reshape(8, S, J).transpose(0, 2, 1).reshape(64, S)
    ).astype(np.float32)
    in_maps = []
    for c in range(8):
        rows = slice(c * S, (c + 1) * S)
        lmask = (NEG * (1.0 - adj[rows].T)).astype(ml_dtypes.bfloat16)
        m = {
            "x0own": np.ascontiguousarray(x0T[:, rows]),
            "f0T": f0T,
            "f0own": np.ascontiguousarray(f0[rows].T).astype(ml_dtypes.bfloat16),
            "lmaskT": np.ascontiguousarray(lmask),
            "W0": Wbf[0], "W1": Wbf[1], "W2": W2m,
            "WA0": WAs[0], "WA1": WAs[1], "WA2": WAs[2],
        }
        in_maps.append(m)
    return in_maps


def kernel(**inputs):
    if "nc" not in _CACHE:
        _CACHE["nc"] = _build()
    nc = _CACHE["nc"]
    in_maps = build_in_maps(inputs)
    res = run_bass_kernel_spmd(nc, in_maps, core_ids=list(range(8)))
    out = np.concatenate([r["outT"].T for r in res.results], axis=0)
    return np.ascontiguousarray(out, dtype=np.float32)


if __name__ == "__main__":
    rng = np.random.default_rng(0)
    fake = {
        "node_feats": rng.standard_normal((N_NODES, 512), dtype=np.float32),
        "edge_feats": rng.standard_normal((131072, 16), dtype=np.float32),
        "edge_indices": rng.integers(0, N_NODES, (2, 131072)).astype(np.int32),
        "adj": np.maximum(
            (rng.random((N_NODES, N_NODES)) < 0.01).astype(np.float32),
            np.eye(N_NODES, dtype=np.float32),
        ),
    }
    for i, (fin, fout) in enumerate(LAYERS):
        fake[f"W{i}"] = (rng.standard_normal((H, fin, fout)) * 0.05).astype(np.float32)
        fake[f"a{i}"] = (rng.standard_normal((H, 2, fout)) * 0.05).astype(np.float32)
    o = kernel(**fake)
    print("kernel output", o.shape, o.dtype, np.abs(o).mean())


# revision 32
# speedup vs baseline: 1.3701x; 1.0299x over previous
"""Dense 3-layer GAT on 8 TRN2 NeuronCores (v2).

Sharding: each core owns 512 query nodes (rows of the attention score
matrix). Per layer, each core computes h = x @ W for its own nodes,
AllGathers h (bf16), then computes its 512-query slab of masked-softmax
attention and the attended output.

v2 structure (vs v1):
- adjacency mask folded into the logits as an additive -30000 logmask,
  applied during the score computation on DVE (2 DVE ops + 1 ACT exp
  per 128x512 block); no separate mask multiply.
- softmax denominators via an all-ones [128,128] stationary matmul that
  accumulates broadcast row-sums in PSUM (L0/L1); for L2 a ones column
  is appended to the gathered h so the row-sum falls out of the same
  matmul that computes the attended output.
- f = x @ (W@a) computed with WA as the stationary operand (out [8,S]
  directly in f^T layout, bf16); layer 0 computes f for ALL nodes
  locally from a replicated full x (no layer-0 f AllGather).
- layer boundaries software-pipelined: next layer's f/h matmuls are
  emitted right after the last head's eviction with the AllGathers
  interleaved, so gather latency hides under matmul streams.
- hg tiles are prefetched on the sync DMA queue several key-blocks
  ahead; normalization reads attention PSUM directly.
"""

from contextlib import ExitStack

import numpy as np
import ml_dtypes

import concourse.mybir as mybir
import concourse.tile as tile
from concourse import bacc
from concourse.bass_utils import run_bass_kernel_spmd
from concourse.masks import make_identity

P = 128
N_NODES = 4096
S = 512                    # nodes per core
NB = N_NODES // P          # 32 global key blocks
H = 4
J = 2 * H                  # f rows (src/dst per head)
LAYERS = [(512, 512), (2048, 512), (2048, 64)]
F32 = mybir.dt.float32
BF16 = mybir.dt.bfloat16
AF = mybir.ActivationFunctionType
ALU = mybir.AluOpType
NEG = -30000.0
HG_AHEAD = 4               # hg prefetch depth (key blocks)

_CACHE = {}


def _build():
    nc = bacc.Bacc("TRN2", target_bir_lowering=False, debug=False, num_devices=8)

    x0own_d = nc.dram_tensor("x0own", [512, S], BF16, kind="ExternalInput")
    f0T_d = nc.dram_tensor("f0T", [64, S], F32, kind="ExternalInput")
    f0own_d = nc.dram_tensor("f0own", [J, S], BF16, kind="ExternalInput")
    lmask_d = nc.dram_tensor("lmaskT", [N_NODES, S], BF16, kind="ExternalInput")
    W_d = []
    WA_d = []
    for li, (fin, fout) in enumerate(LAYERS):
        wshape = [H, fin, fout] if li < 2 else [fin, H * 64]
        W_d.append(nc.dram_tensor(f"W{li}", wshape, BF16, kind="ExternalInput"))
        WA_d.append(nc.dram_tensor(f"WA{li}", [fin, J], BF16, kind="ExternalInput"))
    outT_d = nc.dram_tensor("outT", [H * 64, S], F32, kind="ExternalOutput")

    with tile.TileContext(nc) as tc:
        with ExitStack() as ctx:
            constp = ctx.enter_context(tc.tile_pool(name="const", bufs=1))
            lmp = ctx.enter_context(tc.tile_pool(name="lmp", bufs=1))
            x0p = ctx.enter_context(tc.tile_pool(name="x0p", bufs=8))
            xop = ctx.enter_context(tc.tile_pool(name="xop", bufs=4))
            xbp = ctx.enter_context(tc.tile_pool(name="xbp", bufs=32))
            wtp = ctx.enter_context(tc.tile_pool(name="wt", bufs=17))
            wap = ctx.enter_context(tc.tile_pool(name="wap", bufs=17))
            hsp = ctx.enter_context(tc.tile_pool(name="hsp", bufs=4))
            hgp = ctx.enter_context(tc.tile_pool(name="hgp", bufs=8))
            fp = ctx.enter_context(tc.tile_pool(name="fp", bufs=2))
            fsrcp = ctx.enter_context(tc.tile_pool(name="fsrcp", bufs=5))
            zp = ctx.enter_context(tc.tile_pool(name="zp", bufs=3))
            stp = ctx.enter_context(tc.tile_pool(name="stp", bufs=6))
            rcp = ctx.enter_context(tc.tile_pool(name="rcp", bufs=2))
            evp = ctx.enter_context(tc.tile_pool(name="evp", bufs=2))
            psO = ctx.enter_context(tc.tile_pool(name="psO", bufs=4, space="PSUM"))
            psR = ctx.enter_context(tc.tile_pool(name="psR", bufs=2, space="PSUM"))
            psA = ctx.enter_context(tc.tile_pool(name="psA", bufs=2, space="PSUM"))
            dr = ctx.enter_context(tc.tile_pool(name="dram", bufs=1, space="DRAM"))

            identb = constp.tile([64, 64], F32, tag="identb")
            make_identity(nc, identb[:])
            ones_r = constp.tile([1, P], BF16, tag="ones_r")
            nc.any.memset(ones_r[:], 1.0)
            ones_rf = constp.tile([1, P], F32, tag="ones_rf")
            nc.any.memset(ones_rf[:], 1.0)
            ones128 = constp.tile([P, P], BF16, tag="ones128")
            nc.any.memset(ones128[:], 1.0)

            # warmup collective to absorb ncfw first-call staging; emitted
            # first so it completes before the first real gather is needed
            warm_in = dr.tile([P, 4], F32, tag="warm_in")
            warm_out = dr.tile([8, P, 4], F32, tag="warm_out", addr_space="Shared")
            warm_sb = constp.tile([P, 4], F32, tag="warm_sb")
            nc.any.memset(warm_sb[:], 0.0)
            nc.sync.dma_start(warm_in[:], warm_sb[:])
            nc.gpsimd.collective_compute(
                "AllGather", ALU.bypass,
                replica_groups=[list(range(8))],
                ins=[warm_in[:].opt()], outs=[warm_out[:].opt()],
            )

            # layer-0 own x (bf16), 4 fin-chunks [128, 512]
            x0own = []
            for kb in range(4):
                t = xop.tile([P, S], BF16, tag="x0own")
                nc.sync.dma_start(t[:], x0own_d[kb * P:(kb + 1) * P, :])
                x0own.append(t)

            # shared DRAM buffers for the gathers
            # L0: per-head single gathers; L1/L2: per-pair gathers
            agh0_in = [
                dr.tile([S, 512], BF16, tag=f"ag0i{h}", name=f"ag0i{h}")
                for h in range(H)
            ]
            agh0_out = [
                dr.tile([8, S, 512], BF16, tag=f"ag0o{h}", name=f"ag0o{h}",
                        addr_space="Shared")
                for h in range(H)
            ]
            agh_in = {}
            agh_out = {}
            agf_in = {}
            agf_out = {}
            for li in (1, 2):
                fout = LAYERS[li][1]
                for pp in range(2):
                    agh_in[(li, pp)] = dr.tile(
                        [2, S, fout], BF16, tag=f"aghi{li}_{pp}",
                        name=f"aghi{li}_{pp}")
                    agh_out[(li, pp)] = dr.tile(
                        [8, 2, S, fout], BF16, tag=f"agho{li}_{pp}",
                        name=f"agho{li}_{pp}", addr_space="Shared")
                agf_in[li] = dr.tile([J, S], F32, tag=f"agfi{li}",
                                     name=f"agfi{li}")
                agf_out[li] = dr.tile([8, J, S], F32, tag=f"agfo{li}",
                                      name=f"agfo{li}", addr_space="Shared")

            # L2 hg tiles with a preset ones column (col 64)
            hg65 = []
            for i in range(6):
                t = hgp.tile([P, 65], BF16, tag=f"hg65_{i}")
                nc.any.memset(t[:, 64:65], 1.0)
                hg65.append(t)

            def load_w_head(li, h):
                """Load one head's W tiles (li < 2), just-in-time."""
                fin, fout = LAYERS[li]
                KB = fin // P
                tiles = []
                for kb in range(KB):
                    t = wtp.tile([P, fout], BF16, tag="wt",
                                 name=f"w{li}_{h}_{kb}")
                    nc.sync.dma_start(
                        t[:], W_d[li][h, kb * P:(kb + 1) * P, :])
                    tiles.append(t)
                return tiles

            def load_w2():
                KB = LAYERS[2][0] // P
                w = []
                for kb in range(KB):
                    t = wtp.tile([P, H * 64], BF16, tag="wt2",
                                 name=f"w2_{kb}")
                    nc.sync.dma_start(t[:], W_d[2][kb * P:(kb + 1) * P, :])
                    w.append(t)
                return w

            def load_wa_tiles(li):
                fin = LAYERS[li][0]
                KB = fin // P
                tiles = []
                for kb in range(KB):
                    t = wap.tile([P, J], BF16, tag="wa", name=f"wa{li}_{kb}")
                    nc.sync.dma_start(t[:], WA_d[li][kb * P:(kb + 1) * P, :])
                    tiles.append(t)
                return tiles

            # ---------------- layer 0 front end ----------------
            def l0_head_mm(h):
                w0h = load_w_head(0, h)
                for b in range(4):
                    ph = psA.tile([P, S], F32, tag="pa", name=f"ph0_{h}_{b}")
                    for kb in range(4):
                        nc.tensor.matmul(
                            ph[:, 0:512],
                            x0own[kb][:, b * P:(b + 1) * P],
                            w0h[kb][:],
                            start=(kb == 0), stop=(kb == 3),
                        )
                    hsb = hsp.tile([P, 512], BF16, tag="hsb")
                    nc.scalar.copy(hsb[:], ph[:, 0:512])
                    nc.sync.dma_start(agh0_in[h][b * P:(b + 1) * P, :], hsb[:])

            def l0_gather(h):
                nc.gpsimd.collective_compute(
                    "AllGather", ALU.bypass,
                    replica_groups=[list(range(8))],
                    ins=[agh0_in[h][:].opt()], outs=[agh0_out[h][:].opt()],
                )

            # layer-0 f computed host-side; load both layouts
            fsb_own0 = fp.tile([J, S], BF16, tag="fsb8", name="fsb_own0")
            nc.sync.dma_start(fsb_own0[:], f0own_d[:])
            fgat0 = fp.tile([64, S], F32, tag="fgat", name="fgat0")
            nc.sync.dma_start(fgat0[:], f0T_d[:])

            # head h-matmuls, each head's gather starts ASAP
            for h in range(H):
                l0_head_mm(h)
                l0_gather(h)

            # resident logmask (transposed slab), bf16, [key m, own query n];
            # loaded after the layer-0 h DMAs so it doesn't delay the gathers
            lmask_res = lmp.tile([P, NB, S], BF16, tag="lmask")
            nc.sync.dma_start(
                lmask_res[:], lmask_d[:].rearrange("(nb p) n -> p nb n", p=P)
            )

            def fdst_prep(li, fgat_src):
                """fgat_src: [64, 512] bf16 tile view (r-major, j-minor rows).
                Returns fT_sb [128, 4, 8, J] f32 with
                fT_sb[ml, mh, r, j] = f[j][r*512 + mh*128 + ml]."""
                ptd = psA.tile([P, S], F32, tag="pa", name=f"ptd{li}")
                for mh in range(4):
                    nc.tensor.transpose(
                        ptd[:, mh * 64:(mh + 1) * 64],
                        fgat_src[:, mh * P:(mh + 1) * P],
                        identb[:],
                    )
                fT_sb = fp.tile([P, 4, 8, J], F32, tag="fdst", name=f"fdst{li}")
                nc.scalar.copy(
                    fT_sb[:], ptd[:, 0:256]
                    .rearrange("p (mh rj) -> p mh rj", mh=4)
                    .rearrange("p mh (r j) -> p mh r j", r=8)
                )
                return fT_sb

            def fsrc_bcast(li, fsb8, h):
                fr = fp.tile([1, S], BF16, tag="fr", name=f"fr{li}_{h}")
                nc.sync.dma_start(fr[:], fsb8[2 * h:2 * h + 1, :])
                pb = psA.tile([P, S], F32, tag="pa", name=f"pb{li}_{h}")
                nc.tensor.matmul(pb[:], ones_r[:], fr[:],
                                 start=True, stop=True)
                t = fsrcp.tile([P, S], BF16, tag="fsrcb", name=f"fsrcb{li}_{h}")
                nc.scalar.copy(t[:], pb[:])
                return t

            fT_sb0 = fdst_prep(0, fgat0)
            fsrcb0 = [fsrc_bcast(0, fsb_own0, h) for h in range(H)]

            # ---------------- per-layer attention + next-layer front end ----
            def hg_load(li, h, mb):
                """Issue DMA for gathered h tile of key block mb, head h."""
                r, bsub = mb // 4, mb % 4
                if li == 0:
                    t = hgp.tile([P, 512], BF16, tag="hg", name=f"hg{li}")
                    nc.sync.dma_start(
                        t[:], agh0_out[h][r, bsub * P:(bsub + 1) * P, :])
                elif li == 1:
                    t = hgp.tile([P, 512], BF16, tag="hg", name=f"hg{li}")
                    nc.sync.dma_start(
                        t[:],
                        agh_out[(1, h // 2)][r, h % 2, bsub * P:(bsub + 1) * P, :])
                else:
                    t = hg65[(h * NB + mb) % 6]
                    nc.sync.dma_start(
                        t[:, 0:64],
                        agh_out[(2, h // 2)][r, h % 2, bsub * P:(bsub + 1) * P, :])
                return t

            def scores_and_po(li, h, fsrcb, fT_sb):
                """Emit score pipeline + attention matmuls for one head.
                Returns (po_tiles, psr_or_None)."""
                fout = LAYERS[li][1]
                nob = 4 if fout == 512 else 1
                po = [
                    psO.tile([P, S], F32, tag="po", name=f"po{li}_{h}_{ob}")
                    for ob in range(nob)
                ]
                if li < 2:
                    prs = psR.tile([P, S], F32, tag="prs", name=f"prs{li}_{h}")
                else:
                    prs = None
                hgq = [hg_load(li, h, mb) for mb in range(HG_AHEAD)]
                for mbg in range(NB // 2):
                    lrp = zp.tile([P, 2, S], BF16, tag="lr")
                    # blend: ~half the blocks compute z+prelu on DVE (mask
                    # folded into the add), the rest use ACT Prelu with a
                    # paired DVE mask-add after, to balance the two engines
                    if mbg % 16 < 9:
                        for i in range(2):
                            mb = 2 * mbg + i
                            z = zp.tile([P, S], BF16, tag="z")
                            nc.vector.scalar_tensor_tensor(
                                z[:], fsrcb[:],
                                fT_sb[:, mb % 4, mb // 4, 2 * h + 1:2 * h + 2],
                                lmask_res[:, mb, :],
                                ALU.add, ALU.add,
                            )
                            nc.vector.scalar_tensor_tensor(
                                lrp[:, i, :], z[:], 0.2, z[:], ALU.mult, ALU.max,
                            )
                        exp_src = lrp
                    else:
                        for i in range(2):
                            mb = 2 * mbg + i
                            nc.scalar.activation(
                                lrp[:, i, :], fsrcb[:], AF.Prelu,
                                bias=fT_sb[:, mb % 4, mb // 4,
                                           2 * h + 1:2 * h + 2],
                                scale=1.0, alpha=0.2,
                            )
                        lrm = zp.tile([P, 2, S], BF16, tag="lrm")
                        nc.vector.tensor_tensor(
                            lrm[:], lrp[:],
                            lmask_res[:, 2 * mbg:2 * mbg + 2, :], ALU.add)
                        exp_src = lrm
                    st = stp.tile([P, 2, S], BF16, tag="st")
                    nc.scalar.activation(st[:], exp_src[:], AF.Exp, bias=0.0,
                                         scale=1.0)
                    for i in range(2):
                        mb = 2 * mbg + i
                        if mb + HG_AHEAD < NB:
                            hgq.append(hg_load(li, h, mb + HG_AHEAD))
                        hg = hgq[mb]
                        s_t = st[:, i, :]
                        if li < 2:
                            for ob in range(nob):
                                nc.tensor.matmul(
                                    po[ob][:, :],
                                    hg[:, ob * P:(ob + 1) * P],
                                    s_t,
                                    start=(mb == 0), stop=(mb == NB - 1),
                                )
                            nc.tensor.matmul(
                                prs[:], ones128[:], s_t,
                                start=(mb == 0), stop=(mb == NB - 1),
                            )
                        else:
                            nc.tensor.matmul(
                                po[0][0:65, :], hg[:], s_t,
                                start=(mb == 0), stop=(mb == NB - 1),
                            )
                return po, prs

            def evict(li, h, po, prs, xb_next):
                """Normalize + ELU for one head; appends bf16 tiles to
                xb_next (L0/L1) or DMAs the final output (L2)."""
                fout = LAYERS[li][1]
                if li < 2:
                    rb = rcp.tile([P, S], F32, tag="rb", name=f"rb{li}_{h}")
                    nc.vector.reciprocal_approx_fast(rb[:], prs[:])
                    for ob in range(4):
                        t0 = evp.tile([P, S], F32, tag="t0")
                        nc.vector.tensor_tensor(t0[:], po[ob][:], rb[:], ALU.mult)
                        em = evp.tile([P, S], F32, tag="em")
                        nc.scalar.activation(em[:], t0[:], AF.Exp, bias=0.0,
                                             scale=1.0)
                        rl = evp.tile([P, S], F32, tag="rl")
                        nc.scalar.activation(rl[:], t0[:], AF.Relu, bias=0.0,
                                             scale=1.0)
                        xbn = xbp.tile([P, S], BF16, tag="xb", name=f"xb{li}")
                        nc.vector.scalar_tensor_tensor(
                            xbn[:], em[:], -1.0, rl[:], ALU.add, ALU.min)
                        xb_next.append(xbn)
                else:
                    # row 64 of po holds the row-sums
                    rs = rcp.tile([1, S], F32, tag="rs2", name=f"rs2_{h}")
                    nc.scalar.copy(rs[:], po[0][64:65, :])
                    rsr = rcp.tile([1, S], F32, tag="rsr2", name=f"rsr2_{h}")
                    nc.vector.reciprocal_approx_fast(rsr[:], rs[:])
                    pbr = psA.tile([P, S], F32, tag="pa", name=f"pbr2_{h}")
                    nc.tensor.matmul(pbr[0:64, :], ones_rf[:, 0:64], rsr[:],
                                     start=True, stop=True)
                    rbs = rcp.tile([64, S], F32, tag="rbs2", name=f"rbs2_{h}")
                    nc.scalar.copy(rbs[:], pbr[0:64, :])
                    t0 = evp.tile([64, S], F32, tag="t02", name="t02")
                    nc.vector.tensor_tensor(t0[:], po[0][0:64, :], rbs[:],
                                            ALU.mult)
                    em = evp.tile([64, S], F32, tag="em2", name="em2")
                    nc.scalar.activation(em[:], t0[:], AF.Exp, bias=0.0, scale=1.0)
                    rl = evp.tile([64, S], F32, tag="rl2", name="rl2")
                    nc.scalar.activation(rl[:], t0[:], AF.Relu, bias=0.0, scale=1.0)
                    x1 = evp.tile([64, S], F32, tag="x12", name="x12")
                    nc.vector.scalar_tensor_tensor(
                        x1[:], em[:], -1.0, rl[:], ALU.add, ALU.min)
                    em2 = evp.tile([64, S], F32, tag="em2", name="em2b")
                    nc.scalar.activation(em2[:], x1[:], AF.Exp, bias=0.0, scale=1.0)
                    rl2 = evp.tile([64, S], F32, tag="rl2", name="rl2b")
                    nc.scalar.activation(rl2[:], x1[:], AF.Relu, bias=0.0,
                                         scale=1.0)
                    x2 = evp.tile([64, S], F32, tag="x12", name="x2b")
                    nc.vector.scalar_tensor_tensor(
                        x2[:], em2[:], -1.0, rl2[:], ALU.add, ALU.min)
                    nc.sync.dma_start(outT_d[h * 64:(h + 1) * 64, :], x2[:])

            def emit_f_gather(li, psf):
                """Copy accumulated f psum out, start the f AllGather."""
                fsb8 = fp.tile([J, S], F32, tag="fsb8f", name=f"fsb8_{li}")
                nc.scalar.copy(fsb8[:], psf[0:J, :])
                nc.sync.dma_start(agf_in[li][:], fsb8[:])
                fsb8b = fp.tile([J, S], BF16, tag="fsb8", name=f"fsb8b_{li}")
                nc.vector.tensor_copy(fsb8b[:], fsb8[:])
                nc.gpsimd.collective_compute(
                    "AllGather", ALU.bypass,
                    replica_groups=[list(range(8))],
                    ins=[agf_in[li][:].opt()], outs=[agf_out[li][:].opt()],
                )
                return fsb8b

            # L2 f accumulated incrementally as L1 eviction frees x chunks
            f2_state = {}

            def f2_partial(xb_cur, upto_kb):
                if "wa" not in f2_state:
                    f2_state["wa"] = load_wa_tiles(2)
                    f2_state["psf"] = psA.tile([P, S], F32, tag="pa",
                                               name="psf2")
                    f2_state["kb"] = 0
                for kb in range(f2_state["kb"], upto_kb):
                    nc.tensor.matmul(
                        f2_state["psf"][0:J, :], f2_state["wa"][kb][:],
                        xb_cur[kb][:],
                        start=(kb == 0), stop=(kb == 15),
                    )
                f2_state["kb"] = upto_kb

            def next_front_end(li, xb_cur):
                """f + h matmuls and gathers for layer li (1 or 2), reading
                xb_cur (16 bf16 [128,512] fin-chunk tiles)."""
                fin, fout = LAYERS[li]
                KB = fin // P
                if li == 1:
                    wa = load_wa_tiles(li)
                    psf = psA.tile([P, S], F32, tag="pa", name=f"psf{li}")
                    for kb in range(KB):
                        nc.tensor.matmul(
                            psf[0:J, :], wa[kb][:], xb_cur[kb][:],
                            start=(kb == 0), stop=(kb == KB - 1),
                        )
                    fsb8b = emit_f_gather(li, psf)
                    # h matmuls with just-in-time weight loads, gathers per
                    # head pair
                    for h in range(H):
                        wh = load_w_head(li, h)
                        for b in range(4):
                            ph = psA.tile([P, S], F32, tag="pa",
                                          name=f"ph{li}_{h}_{b}")
                            for kb in range(KB):
                                nc.tensor.matmul(
                                    ph[:, 0:fout],
                                    xb_cur[kb][:, b * P:(b + 1) * P],
                                    wh[kb][:],
                                    start=(kb == 0), stop=(kb == KB - 1),
                                )
                            hsb = hsp.tile([P, fout], BF16, tag="hsb")
                            nc.scalar.copy(hsb[:], ph[:, 0:fout])
                            nc.sync.dma_start(
                                agh_in[(li, h // 2)][h % 2,
                                                     b * P:(b + 1) * P, :],
                                hsb[:])
                        if h % 2 == 1:
                            nc.gpsimd.collective_compute(
                                "AllGather", ALU.bypass,
                                replica_groups=[list(range(8))],
                                ins=[agh_in[(li, h // 2)][:].opt()],
                                outs=[agh_out[(li, h // 2)][:].opt()],
                            )
                    return fsb8b
                # L2: f was accumulated during L1 attention; gather it first
                f2_partial(xb_cur, 16)
                fsb8b = emit_f_gather(li, f2_state["psf"])
                w = load_w2()
                # all 4 heads in one 256-wide stream per (b, kb)
                for b in range(4):
                    ph = psA.tile([P, S], F32, tag="pa", name=f"ph2_{b}")
                    for kb in range(KB):
                        nc.tensor.matmul(
                            ph[:, 0:256],
                            xb_cur[kb][:, b * P:(b + 1) * P],
                            w[kb][:],
                            start=(kb == 0), stop=(kb == KB - 1),
                        )
                    hsb = hsp.tile([P, 256], BF16, tag="hsb2")
                    nc.scalar.copy(hsb[:], ph[:, 0:256])
                    for h in range(H):
                        nc.sync.dma_start(
                            agh_in[(2, h // 2)][h % 2,
                                                b * P:(b + 1) * P, :],
                            hsb[:, h * 64:(h + 1) * 64])
                for pp in range(2):
                    nc.gpsimd.collective_compute(
                        "AllGather", ALU.bypass,
                        replica_groups=[list(range(8))],
                        ins=[agh_in[(2, pp)][:].opt()],
                        outs=[agh_out[(2, pp)][:].opt()],
                    )
                return fsb8b

            def layer_attention(li, fsrcb, fT_sb, cbs=None):
                """Software-pipelined heads: S(0) S(1) E(0) S(2) E(1) S(3)
                E(2) E(3); returns xb_next. cbs maps an evicted head index
                to a callback(xb_next) emitted right after that eviction."""
                xb_next = []
                pend = []

                def _evict_one():
                    hh, ppo, pprs = pend.pop(0)
                    evict(li, hh, ppo, pprs, xb_next)
                    if cbs and hh in cbs:
                        cbs[hh](xb_next)

                for h in range(H):
                    po, prs = scores_and_po(li, h, fsrcb[h], fT_sb)
                    pend.append((h, po, prs))
                    if len(pend) >= 2:
                        _evict_one()
                while pend:
                    _evict_one()
                return xb_next

            # ---- run the three layers ----
            xb1 = layer_attention(0, fsrcb0, fT_sb0)

            fsb8_1 = next_front_end(1, xb1)
            fgat1 = fp.tile([64, S], F32, tag="fgat", name="fgat1")
            nc.sync.dma_start(
                fgat1[:], agf_out[1][:].rearrange("r j m -> (r j) m"))
            fT_sb1 = fdst_prep(1, fgat1)
            fsrcb1 = [fsrc_bcast(1, fsb8_1, h) for h in range(H)]
            xb2 = layer_attention(
                1, fsrcb1, fT_sb1,
                cbs={1: lambda xb: f2_partial(xb, 8),
                     2: lambda xb: f2_partial(xb, 12),
                     3: lambda xb: f2_partial(xb, 16)})

            fsb8_2 = next_front_end(2, xb2)
            fgat2 = fp.tile([64, S], F32, tag="fgat", name="fgat2")
            nc.sync.dma_start(
                fgat2[:], agf_out[2][:].rearrange("r j m -> (r j) m"))
            fT_sb2 = fdst_prep(2, fgat2)
            fsrcb2 = [fsrc_bcast(2, fsb8_2, h) for h in range(H)]
            layer_attention(2, fsrcb2, fT_sb2)

    nc.compile()
    return nc


def build_in_maps(inputs):
    node_feats = np.asarray(inputs["node_feats"], dtype=np.float32)
    adj = np.asarray(inputs["adj"], dtype=np.float32)
    Ws = [np.asarray(inputs[f"W{i}"], dtype=np.float32) for i in range(3)]
    As = [np.asarray(inputs[f"a{i}"], dtype=np.float32) for i in range(3)]

    WAs = []
    WAs64 = []
    for W, a in zip(Ws, As):
        wa64 = np.einsum(
            "hfo,hjo->fhj", W.astype(np.float64), a.astype(np.float64)
        ).reshape(W.shape[1], J)
        WAs64.append(wa64)
        WAs.append(np.ascontiguousarray(wa64.astype(ml_dtypes.bfloat16)))
    Wbf = [Ws[0].astype(ml_dtypes.bfloat16), Ws[1].astype(ml_dtypes.bfloat16)]
    # L2 weights merged across heads: [fin, H*64]
    W2m = np.ascontiguousarray(
        np.transpose(Ws[2], (1, 0, 2)).reshape(Ws[2].shape[1], H * 64)
    ).astype(ml_dtypes.bfloat16)

    x0T = np.ascontiguousarray(node_feats.T).astype(ml_dtypes.bfloat16)
    # layer-0 f = x @ (W0@a0) for all nodes, in the gather layout
    # f0T[(r*8+j), c] = f0[r*512+c, j]
    f0 = (node_feats.astype(np.float64) @ WAs64[0]).astype(np.float32)  # [N, J]
    f0T = np.ascontiguousarray(
        f0.reshape(8, S, J).transpose(0, 2, 1).reshape(64, S)
    ).astype(np.float32)
    in_maps = []
    for c in range(8):
        rows = slice(c * S, (c + 1) * S)
        lmask = (NEG * (1.0 - adj[rows].T)).astype(ml_dtypes.bfloat16)
        m = {
            "x0own": np.ascontiguousarray(x0T[:, rows]),
            "f0T": f0T,
            "f0own": np.ascontiguousarray(f0[rows].T).astype(ml_dtypes.bfloat16),
            "lmaskT": np.ascontiguousarray(lmask),
            "W0": Wbf[0], "W1": Wbf[1], "W2": W2m,
            "WA0": WAs[0], "WA1": WAs[1], "WA2": WAs[2],
        }
        in_maps.append(m)
    return in_maps


def kernel(**inputs):
    if "nc" not in _CACHE:
        _CACHE["nc"] = _build()
    nc = _CACHE["nc"]
    in_maps = build_in_maps(inputs)
    res = run_bass_kernel_spmd(nc, in_maps, core_ids=list(range(8)))
    out = np.concatenate([r["outT"].T for r in res.results], axis=0)
    return np.ascontiguousarray(out, dtype=np.float32)


if __name__ == "__main__":
    rng = np.random.default_rng(0)
    fake = {
        "node_feats": rng.standard_normal((N_NODES, 512), dtype=np.float32),
        "edge_feats": rng.standard_normal((131072, 16), dtype=np.float32),
        "edge_indices": rng.integers(0, N_NODES, (2, 131072)).astype(np.int32),
        "adj": np.maximum(
            (rng.random((N_NODES, N_NODES)) < 0.01).astype(np.float32),
            np.eye(N_NODES, dtype=np.float32),
        ),
    }
    for i, (fin, fout) in enumerate(LAYERS):
        fake[f"W{i}"] = (rng.standard_normal((H, fin, fout)) * 0.05).astype(np.float32)
        fake[f"a{i}"] = (rng.standard_normal((H, 2, fout)) * 0.05).astype(np.float32)
    o = kernel(**fake)
    print("kernel output", o.shape, o.dtype, np.abs(o).mean())
